# revision 5
# baseline (speedup 1.0000x reference)
"""Cosine-similarity kernel for trn2: out = l2norm_rows(x) @ l2norm_rows(W).

x: [65536, 512] f32, W: [512, 462] f32 -> out: [65536, 462] f32.

Strategy (data-parallel over 8 cores, batch-sharded x, replicated W):
  The host hands each core x^T for its batch shard (layout marshaling
  only) so the contraction dim (in_dim) lands on SBUF partitions.

  Per core (8192 batch rows), per group of 1024 rows:
  - GEMM in NATURAL output layout: stationary = x^T tile [128K, 128b]
    (a direct slice of the x^T SBUF tile, no transpose), moving =
    normalized W chunk [128K, 462o], f32r.  PSUM out [128b, 462o].
  - Row sumsq: ACT squares x^T (f32r), ones[128,1]-stationary matmuls
    reduce over partitions -> ssq [1, 1024] row.
  - K=1 transpose matmuls flip ssq to partition-major [128, 8], then
    sqrt(ssq+eps) on ACT and reciprocal on DVE run on a tiny tile.
  - Eviction fuses the normalize: Copy-with-per-partition-scale on ACT
    (and tensor_scalar_mul on DVE for half the tiles) moves PSUM->SBUF
    scaled by 1/||x_row||.
  - Output DMA'd back in natural layout (no host transpose on output).
"""

from contextlib import ExitStack

import numpy as np

import concourse.bass as bass
import concourse.mybir as mybir
import concourse.tile as tile
from concourse import bacc, bass_utils
from concourse.bass import ds

N_CORES = 8
B = 65536
B_PER = B // N_CORES          # 8192 batch rows per core
IN_DIM = 512
OUT_DIM = 462
EPS = 1e-12
P = 128
KC = IN_DIM // P              # 4 contraction chunks
GROUP_COLS = 1024             # batch rows per group (2 MB in)
JT = GROUP_COLS // P          # 8 b-tiles of 128 rows per group
N_GROUPS = B_PER // GROUP_COLS

F32 = mybir.dt.float32
F32R = mybir.dt.float32r


def _build_bass():
    nc = bacc.Bacc("TRN2", debug=False, num_devices=N_CORES)
    xt_d = nc.dram_tensor("xt", [IN_DIM, B_PER], F32R, kind="ExternalInput").ap()
    w_d = nc.dram_tensor("w", [IN_DIM, OUT_DIM], F32, kind="ExternalInput").ap()
    o_d = nc.dram_tensor("o", [B_PER, OUT_DIM], F32, kind="ExternalOutput").ap()

    with ExitStack() as ctx:
        tc = ctx.enter_context(tile.TileContext(nc))

        singles = ctx.enter_context(tc.tile_pool(name="singles", bufs=1))
        xpool = ctx.enter_context(tc.tile_pool(name="xin", bufs=2))
        sqpool = ctx.enter_context(tc.tile_pool(name="sq", bufs=2))
        opool = ctx.enter_context(tc.tile_pool(name="oout", bufs=2))
        stats = ctx.enter_context(tc.tile_pool(name="stats", bufs=2))
        psum_o = ctx.enter_context(tc.tile_pool(name="psum_o", bufs=4, space="PSUM"))
        psum_s = ctx.enter_context(tc.tile_pool(name="psum_s", bufs=2, space="PSUM"))
        psum_t = ctx.enter_context(tc.tile_pool(name="psum_t", bufs=2, space="PSUM"))

        zero_bias = singles.tile([P, 1], F32)
        nc.vector.memset(zero_bias, 0.0)
        ones_f = singles.tile([P, 1], F32)
        nc.vector.memset(ones_f, 1.0)
        ones_k = singles.tile([P, 1], F32R)   # reduce-over-partitions stationary
        nc.vector.tensor_copy(out=ones_k, in_=ones_f)
        one1 = singles.tile([1, 1], F32)      # K=1 transpose moving operand
        nc.vector.memset(one1, 1.0)
        eps_bias = singles.tile([P, 1], F32)
        nc.vector.memset(eps_bias, EPS)

        # ---- W normalization (once) ----
        w_sb = singles.tile([P, KC, OUT_DIM], F32)
        nc.sync.dma_start(w_sb, w_d.rearrange("(c p) o -> p c o", p=P))
        wsq = singles.tile([P, KC, OUT_DIM], F32)  # scratch squares
        wssq = singles.tile([P, KC], F32)
        for c in range(KC):
            nc.scalar.activation(
                out=wsq[:, c, :],
                in_=w_sb[:, c, :],
                func=mybir.ActivationFunctionType.Square,
                bias=zero_bias,
                accum_out=wssq[:, c : c + 1],
            )
        nc.vector.tensor_scalar_max(wssq, wssq, EPS)
        nc.scalar.activation(
            out=wssq, in_=wssq, func=mybir.ActivationFunctionType.Sqrt, bias=zero_bias
        )
        wrs = singles.tile([P, KC], F32)
        nc.vector.reciprocal(wrs, wssq)
        # f32r so the PE matmul runs at 1 cycle/row; producer rounds to f32r
        wn_sb = singles.tile([P, KC, OUT_DIM], F32R)
        for c in range(KC):
            nc.vector.tensor_scalar_mul(wn_sb[:, c, :], w_sb[:, c, :], wrs[:, c : c + 1])

        # ---- main loop ----
        xt_v = xt_d.rearrange("(c p) b -> p c b", p=P)  # [128, KC, B_PER]
        for g in range(N_GROUPS):
            b0 = g * GROUP_COLS
            x_sb = xpool.tile([P, KC, GROUP_COLS], F32R)
            nc.sync.dma_start(x_sb, xt_v[:, :, ds(b0, GROUP_COLS)])

            # --- row sumsq: squares on ACT, partition-reduce on PE ---
            xsq = sqpool.tile([P, KC, GROUP_COLS], F32R)
            nc.scalar.activation(
                out=xsq,
                in_=x_sb,
                func=mybir.ActivationFunctionType.Square,
                bias=zero_bias,
            )
            s_row = stats.tile([1, GROUP_COLS], F32)
            for h in range(GROUP_COLS // 512):
                ps_ssq = psum_s.tile([1, 512], F32)
                for c in range(KC):
                    nc.tensor.matmul(
                        ps_ssq,
                        lhsT=ones_k[:, :],
                        rhs=xsq[:, c, ds(h * 512, 512)],
                        start=(c == 0),
                        stop=(c == KC - 1),
                    )
                nc.scalar.activation(
                    out=s_row[:, ds(h * 512, 512)],
                    in_=ps_ssq,
                    func=mybir.ActivationFunctionType.Copy,
                )

            # --- transpose ssq to partition-major via K=1 matmuls ---
            ps_s = psum_t.tile([P, JT], F32)
            for j in range(JT):
                nc.tensor.matmul(
                    ps_s[:, j : j + 1],
                    lhsT=s_row[:, ds(j * P, P)],
                    rhs=one1[:, :],
                )
            sq_s = stats.tile([P, JT], F32)
            nc.scalar.activation(
                out=sq_s,
                in_=ps_s,
                func=mybir.ActivationFunctionType.Sqrt,
                bias=eps_bias,
            )
            s_col = stats.tile([P, JT], F32)
            nc.vector.reciprocal(s_col, sq_s)

            # --- GEMM: natural out tiles = x^T tile (stationary) vs wn ---
            ot_a = opool.tile([P, JT // 2, OUT_DIM], F32)  # even j  (ACT evict)
            ot_b = opool.tile([P, JT // 2, OUT_DIM], F32)  # odd j   (DVE evict)
            for j in range(JT):
                po = psum_o.tile([P, OUT_DIM], F32)
                for c in range(KC):
                    nc.tensor.matmul(
                        po,
                        lhsT=x_sb[:, c, ds(j * P, P)],
                        rhs=wn_sb[:, c, :],
                        start=(c == 0),
                        stop=(c == KC - 1),
                    )
                # fused normalize: per-partition scale while evicting PSUM
                if j % 2 == 0:
                    nc.scalar.activation(
                        out=ot_a[:, j // 2, :],
                        in_=po,
                        func=mybir.ActivationFunctionType.Copy,
                        scale=s_col[:, j : j + 1],
                    )
                else:
                    nc.vector.tensor_scalar_mul(
                        ot_b[:, j // 2, :], po, s_col[:, j : j + 1]
                    )

            # natural-layout group store: row = b0 + j*128 + p
            dst_a = bass.AP(
                tensor=o_d.tensor,
                offset=b0 * OUT_DIM,
                ap=[[OUT_DIM, P], [2 * P * OUT_DIM, JT // 2], [1, OUT_DIM]],
            )
            nc.scalar.dma_start(dst_a, ot_a)
            dst_b = bass.AP(
                tensor=o_d.tensor,
                offset=(b0 + P) * OUT_DIM,
                ap=[[OUT_DIM, P], [2 * P * OUT_DIM, JT // 2], [1, OUT_DIM]],
            )
            nc.scalar.dma_start(dst_b, ot_b)

    nc.compile()
    return nc


_NC_CACHE = None
LAST_RESULTS = None  # BassKernelResults of the most recent run (for profiling)


def kernel(x: np.ndarray, W: np.ndarray) -> np.ndarray:
    global _NC_CACHE, LAST_RESULTS
    if _NC_CACHE is None:
        _NC_CACHE = _build_bass()
    nc = _NC_CACHE

    x = np.asarray(x, dtype=np.float32)
    W = np.ascontiguousarray(np.asarray(W, dtype=np.float32))
    in_maps = []
    for i in range(N_CORES):
        shard = np.ascontiguousarray(x[i * B_PER : (i + 1) * B_PER].T)
        in_maps.append({"xt": shard, "w": W})
    res = bass_utils.run_bass_kernel_spmd(nc, in_maps, core_ids=list(range(N_CORES)))
    LAST_RESULTS = res
    out = np.concatenate([np.asarray(r["o"]) for r in res.results], axis=0)
    return out


# revision 9
# speedup vs baseline: 1.2437x; 1.2437x over previous
"""Cosine-similarity kernel for trn2: out = l2norm_rows(x) @ l2norm_rows(W).

x: [65536, 512] f32, W: [512, 462] f32 -> out: [65536, 462] f32.

Strategy (data-parallel over 8 cores, batch-sharded x, replicated W):
  The host hands each core x^T for its batch shard (layout marshaling
  only) so the contraction dim (in_dim) lands on SBUF partitions.

  Per core (8192 batch rows), per group of 1024 rows:
  - GEMM in NATURAL output layout: stationary = x^T tile [128K, 128b]
    (a direct slice of the x^T SBUF tile, no transpose), moving =
    normalized W chunk [128K, 462o], f32r.  PSUM out [128b, 462o].
  - Row sumsq: squares split ACT/DVE, ones[128,1]-stationary matmuls
    reduce over partitions -> ssq [1, 1024] row (DVE-evicted).
  - ssq flipped to partition-major in one shot: SBUF->SBUF DMA shuffle
    [1,1024]->[8,128] (scalar HWDGE queue) + a single eye8 matmul ->
    [128, 8]; then sqrt(ssq+eps) on ACT and reciprocal on DVE run on
    the tiny tile.
  - Eviction fuses the normalize: Copy-with-per-partition-scale (ACT
    for even b-tiles, tensor_scalar_mul on DVE for odd) moves
    PSUM->SBUF scaled by 1/||x_row||.
  - Outputs stored in natural layout via gpsimd SWDGE so the scalar
    and sync queues stay dedicated to s-shuffles / inputs.
"""

from contextlib import ExitStack

import numpy as np

import concourse.bass as bass
import concourse.mybir as mybir
import concourse.tile as tile
from concourse import bacc, bass_utils
from concourse.bass import ds

N_CORES = 8
B = 65536
B_PER = B // N_CORES          # 8192 batch rows per core
IN_DIM = 512
OUT_DIM = 462
EPS = 1e-12
P = 128
KC = IN_DIM // P              # 4 contraction chunks
GROUP_COLS = 1024             # batch rows per group (2 MB in)
JT = GROUP_COLS // P          # 8 b-tiles of 128 rows per group
N_GROUPS = B_PER // GROUP_COLS

F32 = mybir.dt.float32
F32R = mybir.dt.float32r


def _build_bass():
    nc = bacc.Bacc("TRN2", debug=False, num_devices=N_CORES)
    xt_d = nc.dram_tensor("xt", [IN_DIM, B_PER], F32R, kind="ExternalInput").ap()
    w_d = nc.dram_tensor("w", [IN_DIM, OUT_DIM], F32, kind="ExternalInput").ap()
    o_d = nc.dram_tensor("o", [B_PER, OUT_DIM], F32, kind="ExternalOutput").ap()
    eye_d = nc.dram_tensor("eye8", [8, 8], F32, kind="ExternalInput").ap()

    with ExitStack() as ctx:
        tc = ctx.enter_context(tile.TileContext(nc))

        singles = ctx.enter_context(tc.tile_pool(name="singles", bufs=1))
        xpool = ctx.enter_context(tc.tile_pool(name="xin", bufs=3))
        sqpool = ctx.enter_context(tc.tile_pool(name="sq", bufs=2))
        opool = ctx.enter_context(tc.tile_pool(name="oout", bufs=2))
        stats = ctx.enter_context(tc.tile_pool(name="stats", bufs=3))
        psum_o = ctx.enter_context(tc.tile_pool(name="psum_o", bufs=3, space="PSUM"))
        psum_s = ctx.enter_context(tc.tile_pool(name="psum_s", bufs=3, space="PSUM"))
        psum_t = ctx.enter_context(tc.tile_pool(name="psum_t", bufs=2, space="PSUM"))

        zero_bias = singles.tile([P, 1], F32)
        nc.vector.memset(zero_bias, 0.0)
        ones_f = singles.tile([P, 1], F32)
        nc.vector.memset(ones_f, 1.0)
        ones_k = singles.tile([P, 1], F32R)   # reduce-over-partitions stationary
        nc.vector.tensor_copy(out=ones_k, in_=ones_f)
        eps_bias = singles.tile([P, 1], F32)
        nc.vector.memset(eps_bias, EPS)
        eye8 = singles.tile([8, 8], F32)      # transpose moving operand
        nc.sync.dma_start(eye8, eye_d)

        # ---- W normalization (once) ----
        w_sb = singles.tile([P, KC, OUT_DIM], F32)
        nc.sync.dma_start(w_sb, w_d.rearrange("(c p) o -> p c o", p=P))
        wsq = singles.tile([P, KC, OUT_DIM], F32)  # scratch squares
        wssq = singles.tile([P, KC], F32)
        for c in range(KC):
            nc.scalar.activation(
                out=wsq[:, c, :],
                in_=w_sb[:, c, :],
                func=mybir.ActivationFunctionType.Square,
                bias=zero_bias,
                accum_out=wssq[:, c : c + 1],
            )
        nc.vector.tensor_scalar_max(wssq, wssq, EPS)
        nc.scalar.activation(
            out=wssq, in_=wssq, func=mybir.ActivationFunctionType.Sqrt, bias=zero_bias
        )
        wrs = singles.tile([P, KC], F32)
        nc.vector.reciprocal(wrs, wssq)
        # f32r so the PE matmul runs at 1 cycle/row; producer rounds to f32r
        wn_sb = singles.tile([P, KC, OUT_DIM], F32R)
        for c in range(KC):
            nc.vector.tensor_scalar_mul(wn_sb[:, c, :], w_sb[:, c, :], wrs[:, c : c + 1])

        # ---- main loop ----
        xt_v = xt_d.rearrange("(c p) b -> p c b", p=P)  # [128, KC, B_PER]
        for g in range(N_GROUPS):
            b0 = g * GROUP_COLS
            x_sb = xpool.tile([P, KC, GROUP_COLS], F32R)
            nc.sync.dma_start(x_sb, xt_v[:, :, ds(b0, GROUP_COLS)])

            # --- row sumsq: squares split ACT/DVE, partition-reduce on PE ---
            xsq = sqpool.tile([P, KC, GROUP_COLS], F32R)
            nc.scalar.activation(
                out=xsq[:, 0:2, :],
                in_=x_sb[:, 0:2, :],
                func=mybir.ActivationFunctionType.Square,
                bias=zero_bias,
            )
            nc.vector.tensor_mul(xsq[:, 2:4, :], x_sb[:, 2:4, :], x_sb[:, 2:4, :])
            s_row = stats.tile([1, GROUP_COLS], F32)
            for h in range(GROUP_COLS // 512):
                ps_ssq = psum_s.tile([1, 512], F32)
                for c in range(KC):
                    nc.tensor.matmul(
                        ps_ssq,
                        lhsT=ones_k[:, :],
                        rhs=xsq[:, c, ds(h * 512, 512)],
                        start=(c == 0),
                        stop=(c == KC - 1),
                    )
                nc.vector.tensor_copy(out=s_row[:, ds(h * 512, 512)], in_=ps_ssq)

            # --- flip ssq to partition-major: DMA shuffle + eye8 matmul ---
            s8 = stats.tile([8, P], F32)
            nc.scalar.dma_start(s8, s_row)
            ps_s = psum_t.tile([P, JT], F32)
            nc.tensor.matmul(ps_s, lhsT=s8, rhs=eye8)
            sq_s = stats.tile([P, JT], F32)
            nc.scalar.activation(
                out=sq_s,
                in_=ps_s,
                func=mybir.ActivationFunctionType.Sqrt,
                bias=eps_bias,
            )
            s_col = stats.tile([P, JT], F32)
            nc.vector.reciprocal(s_col, sq_s)

            # --- GEMM: natural out tiles = x^T tile (stationary) vs wn ---
            ot_a = opool.tile([P, JT // 2, OUT_DIM], F32)  # even j  (ACT evict)
            ot_b = opool.tile([P, JT // 2, OUT_DIM], F32)  # odd j   (DVE evict)
            for j in range(JT):
                po = psum_o.tile([P, OUT_DIM], F32)
                for c in range(KC):
                    nc.tensor.matmul(
                        po,
                        lhsT=x_sb[:, c, ds(j * P, P)],
                        rhs=wn_sb[:, c, :],
                        start=(c == 0),
                        stop=(c == KC - 1),
                    )
                # fused normalize: per-partition scale while evicting PSUM
                if j % 2 == 0:
                    nc.scalar.activation(
                        out=ot_a[:, j // 2, :],
                        in_=po,
                        func=mybir.ActivationFunctionType.Copy,
                        scale=s_col[:, j : j + 1],
                    )
                else:
                    nc.vector.tensor_scalar_mul(
                        ot_b[:, j // 2, :], po, s_col[:, j : j + 1]
                    )

            # natural-layout group store: row = b0 + j*128 + p (SWDGE)
            dst_a = bass.AP(
                tensor=o_d.tensor,
                offset=b0 * OUT_DIM,
                ap=[[OUT_DIM, P], [2 * P * OUT_DIM, JT // 2], [1, OUT_DIM]],
            )
            nc.gpsimd.dma_start(dst_a, ot_a)
            dst_b = bass.AP(
                tensor=o_d.tensor,
                offset=(b0 + P) * OUT_DIM,
                ap=[[OUT_DIM, P], [2 * P * OUT_DIM, JT // 2], [1, OUT_DIM]],
            )
            nc.gpsimd.dma_start(dst_b, ot_b)

    nc.compile()
    return nc


_NC_CACHE = None
LAST_RESULTS = None  # BassKernelResults of the most recent run (for profiling)


def kernel(x: np.ndarray, W: np.ndarray) -> np.ndarray:
    global _NC_CACHE, LAST_RESULTS
    if _NC_CACHE is None:
        _NC_CACHE = _build_bass()
    nc = _NC_CACHE

    x = np.asarray(x, dtype=np.float32)
    W = np.ascontiguousarray(np.asarray(W, dtype=np.float32))
    in_maps = []
    for i in range(N_CORES):
        shard = np.ascontiguousarray(x[i * B_PER : (i + 1) * B_PER].T)
        in_maps.append({"xt": shard, "w": W, "eye8": np.eye(8, dtype=np.float32)})
    res = bass_utils.run_bass_kernel_spmd(nc, in_maps, core_ids=list(range(N_CORES)))
    LAST_RESULTS = res
    out = np.concatenate([np.asarray(r["o"]) for r in res.results], axis=0)
    return out
